# revision 10
# baseline (speedup 1.0000x reference)
"""Trainium2 Bass kernel for the interval-prediction custom loss.

total = 10*mean((t - (l+u)/2)^2) + 0.1*mean(u-l) + 10*mean(relu(l-u))
        + 0.5*sum(where(pv==0, relu(c-p), relu(p-c)))/N        with c=(l+u)/2

Strategy: pure data parallel over N across 8 NeuronCores; host does only the
tiny final scalar reduction in float64 (same contract as before).

Engine plan (v2). The v1 kernel was DVE+ACT bound (DVE 36.5us busy of a
57us span) because scalar_tensor_tensor ops run in 1x mode. v2 uses only
2x-capable tensor_tensor and 4x-capable tensor_scalar ops on the DVE, and
moves the remaining reductions onto the otherwise-idle PE and the ACT:

  DVE: H = lo + up                  (TT 2x)
       c = 0.5*H, accum -> S_c      (TS 4x)   S_H = 2*S_c
       x = c - p                    (TT 2x)
       e = c - t                    (TT 2x)
       q = v * x                    (TT 2x)
       max(x,0), accum -> S_rx      (TS 4x)
  ACT: Square(e), accum -> S_sq
       Relu(d) from PSUM, accum -> S_vd
  PE:  d = I*lo + (-I)*up -> PSUM          (identity matmuls)
       S_up  += ones^T * up  -> PSUM[1,512] (column sums)
       S_vx  += ones^T * q   -> PSUM[1,512]

Host combine: S_w = 2*S_up - 2*S_c, direction = S_rx - S_vx,
valid = S_vd, center = S_sq.

All five streams are bf16 (int64 pv is 0/1 so the cast is exact); all
on-chip accumulation is fp32. Tile widths are multiples of 512 so every
PE matmul chunk is exactly 512 wide (one PSUM bank) and the [1,512]
column-sum accumulators are fully initialized by their first matmul.
"""

import sys

if "/opt/trn_rl_repo" not in sys.path:
    sys.path.insert(0, "/opt/trn_rl_repo")

import numpy as np

N = 8388608
N_CORES = 8
P = 128
NP_PER_CORE = N // N_CORES            # 1048576
FPL = NP_PER_CORE // P                # 8192 elements per partition lane
TILE_WIDTHS = (512, 512, 1024, 2048, 2048, 1536, 512)
assert sum(TILE_WIDTHS) == FPL
assert all(w % 512 == 0 for w in TILE_WIDTHS)
MM = 512                              # matmul moving chunk / PSUM bank width
DSUB = 1024                           # PSUM d-subtile width (2 banks)

_NC_CACHE = {}


def _dsub_widths(tw):
    """Split a tile width into PSUM d-subtile widths (<= DSUB each)."""
    out = []
    off = 0
    while off < tw:
        w = min(DSUB, tw - off)
        out.append(w)
        off += w
    return out


def _n_dsubs(widths):
    return sum(len(_dsub_widths(tw)) for tw in widths)


def _build(fpl=FPL, widths=TILE_WIDTHS):
    """Build the per-core Bass program (identical on all cores)."""
    from concourse import bacc, mybir
    from concourse.tile import TileContext

    assert sum(widths) == fpl
    n_tiles = len(widths)
    n_dsubs = _n_dsubs(widths)

    f32 = mybir.dt.float32
    bf16 = mybir.dt.bfloat16
    Alu = mybir.AluOpType
    Act = mybir.ActivationFunctionType

    nc = bacc.Bacc(trn_type="TRN2")
    big = nc.declare_dram_parameter("big", [P, 5 * fpl], bf16, isOutput=False)
    # columns: [0:128) identity, [128:256) -identity, [256] ones
    consts = nc.declare_dram_parameter("consts", [P, 257], bf16, isOutput=False)
    # accumulator columns: S_c (n_tiles) | S_rx (n_tiles) | S_sq (n_tiles)
    #                      | S_vd (n_dsubs)
    out = nc.declare_dram_parameter(
        "out", [P, 3 * n_tiles + n_dsubs], f32, isOutput=True
    )
    # row 0: S_up column sums, row 1: S_vx column sums
    sums = nc.declare_dram_parameter("sums", [2, MM], f32, isOutput=True)

    with TileContext(nc) as tc:
        with (
            tc.tile_pool(name="io", bufs=3) as io_pool,
            tc.tile_pool(name="mid", bufs=2) as mid_pool,
            tc.tile_pool(name="acc", bufs=1) as acc_pool,
            tc.tile_pool(name="psd", bufs=2, space="PSUM") as psd_pool,
            tc.tile_pool(name="pss", bufs=1, space="PSUM") as pss_pool,
        ):
            const_t = acc_pool.tile([P, 257], bf16, tag="consts")
            nc.sync.dma_start(out=const_t, in_=consts[:, :])
            ident = const_t[:, 0:128]
            nident = const_t[:, 128:256]
            onesv = const_t[:, 256:257]

            acc_dve = acc_pool.tile([P, 2 * n_tiles], f32, tag="acc_dve")
            acc_act = acc_pool.tile([P, n_tiles + n_dsubs], f32, tag="acc_act")
            sums_up_sb = acc_pool.tile([1, MM], f32, tag="sums_up_sb")
            sums_vx_sb = acc_pool.tile([1, MM], f32, tag="sums_vx_sb")
            ps_up = pss_pool.tile([1, MM], f32, tag="ps_up")
            ps_vx = pss_pool.tile([1, MM], f32, tag="ps_vx")

            off = 0
            dix = 0
            first_up = True
            first_q = True
            n_chunks_left = fpl // MM  # per-stream 512-chunks remaining
            for j, tw in enumerate(widths):
                big_t = io_pool.tile([P, 5, tw], bf16, tag="big", name=f"big{j}")
                src = big[:, off : off + 5 * tw].rearrange("p (s f) -> p s f", s=5)
                # two DMAs: [lo,up] first (enables H/c and all PE work),
                # then [t,p,v] (enables x/e/q)
                nc.sync.dma_start(out=big_t[:, 0:2, :], in_=src[:, 0:2, :])
                nc.sync.dma_start(out=big_t[:, 2:5, :], in_=src[:, 2:5, :])
                off += 5 * tw

                lo = big_t[:, 0, :]
                up = big_t[:, 1, :]
                t_t = big_t[:, 2, :]
                p_t = big_t[:, 3, :]
                v_t = big_t[:, 4, :]

                H = mid_pool.tile([P, tw], bf16, tag="H", name=f"H{j}")
                c = mid_pool.tile([P, tw], bf16, tag="c", name=f"c{j}")
                x = mid_pool.tile([P, tw], bf16, tag="x", name=f"x{j}")
                e = mid_pool.tile([P, tw], bf16, tag="e", name=f"e{j}")
                q = mid_pool.tile([P, tw], bf16, tag="q", name=f"q{j}")
                jk = mid_pool.tile([P, tw], bf16, tag="jk", name=f"jk{j}")
                ja = mid_pool.tile([P, tw], bf16, tag="ja", name=f"ja{j}")

                # --- DVE (all 2x/4x mode ops) ---
                nc.vector.tensor_add(out=H, in0=lo, in1=up)
                nc.vector.tensor_scalar(
                    out=c, in0=H, scalar1=0.5, scalar2=None, op0=Alu.mult,
                    op1=Alu.add, accum_out=acc_dve[:, j : j + 1],
                )
                nc.vector.tensor_sub(out=x, in0=c, in1=p_t)
                nc.vector.tensor_sub(out=e, in0=c, in1=t_t)
                nc.vector.tensor_mul(out=q, in0=v_t, in1=x)
                nc.vector.tensor_scalar(
                    out=jk, in0=x, scalar1=0.0, scalar2=None, op0=Alu.max,
                    op1=Alu.add,
                    accum_out=acc_dve[:, n_tiles + j : n_tiles + j + 1],
                )

                # --- ACT: sum(e^2) ---
                nc.scalar.activation(
                    out=ja, in_=e, func=Act.Square,
                    accum_out=acc_act[:, j : j + 1],
                )

                # --- PE: d = lo - up into PSUM, then ACT relu-sum ---
                doff = 0
                for dw in _dsub_widths(tw):
                    d_ps = psd_pool.tile([P, dw], f32, tag="d_ps", name=f"d{dix}")
                    for ch in range(0, dw, MM):
                        nc.tensor.matmul(
                            d_ps[:, ch : ch + MM],
                            ident,
                            lo[:, doff + ch : doff + ch + MM],
                            start=True, stop=False,
                        )
                    for ch in range(0, dw, MM):
                        nc.tensor.matmul(
                            d_ps[:, ch : ch + MM],
                            nident,
                            up[:, doff + ch : doff + ch + MM],
                            start=False, stop=True,
                        )
                    jd = mid_pool.tile([P, dw], bf16, tag="jd", name=f"jd{dix}")
                    nc.scalar.activation(
                        out=jd, in_=d_ps, func=Act.Relu,
                        accum_out=acc_act[:, n_tiles + dix : n_tiles + dix + 1],
                    )
                    doff += dw
                    dix += 1

                # --- PE: column-sum accumulators ---
                last_tile = j == len(widths) - 1
                for ci, ch in enumerate(range(0, tw, MM)):
                    is_last = last_tile and ci == tw // MM - 1
                    nc.tensor.matmul(
                        ps_up, onesv, up[:, ch : ch + MM],
                        start=first_up, stop=is_last,
                    )
                    first_up = False
                for ci, ch in enumerate(range(0, tw, MM)):
                    is_last = last_tile and ci == tw // MM - 1
                    nc.tensor.matmul(
                        ps_vx, onesv, q[:, ch : ch + MM],
                        start=first_q, stop=is_last,
                    )
                    first_q = False

            assert dix == n_dsubs

            # close the accumulation groups with a no-op-sized final read via
            # copies: ACT copies PSUM column sums into SBUF, then DMA out.
            nc.scalar.activation(out=sums_up_sb[:, :], in_=ps_up, func=Act.Copy)
            nc.scalar.activation(out=sums_vx_sb[:, :], in_=ps_vx, func=Act.Copy)

            nc.sync.dma_start(out=out[:, 0 : 2 * n_tiles], in_=acc_dve)
            nc.sync.dma_start(
                out=out[:, 2 * n_tiles : 3 * n_tiles + n_dsubs], in_=acc_act
            )
            nc.sync.dma_start(out=sums[0:1, :], in_=sums_up_sb)
            nc.sync.dma_start(out=sums[1:2, :], in_=sums_vx_sb)

    nc.compile()
    return nc


def _get_nc():
    key = (FPL, TILE_WIDTHS)
    if key not in _NC_CACHE:
        _NC_CACHE[key] = _build()
    return _NC_CACHE[key]


def _make_consts():
    import ml_dtypes

    bf = ml_dtypes.bfloat16
    consts = np.zeros((P, 257), dtype=bf)
    consts[:, 0:128] = np.eye(P, dtype=bf)
    consts[:, 128:256] = -np.eye(P, dtype=bf)
    consts[:, 256] = bf(1.0)
    return consts


def _shard(inputs, fpl=FPL, widths=TILE_WIDTHS, n_cores=N_CORES):
    import ml_dtypes

    bf = ml_dtypes.bfloat16
    n = n_cores * P * fpl
    pred = np.asarray(inputs["pred"])
    targ = np.asarray(inputs["target"]).reshape(n)
    prev = np.asarray(inputs["prev_pci"]).reshape(n)
    # int64 is unsupported on-device; values are 0/1 so a bf16 cast is exact.
    pv = np.asarray(inputs["pv_values"]).astype(bf).reshape(n)

    lo = pred[:, 0].astype(bf)
    up = pred[:, 1].astype(bf)
    tb = targ.astype(bf)
    pb = prev.astype(bf)

    consts = _make_consts()
    np_per_core = P * fpl

    in_maps = []
    for cix in range(n_cores):
        s = slice(cix * np_per_core, (cix + 1) * np_per_core)
        streams = (
            lo[s].reshape(P, fpl),
            up[s].reshape(P, fpl),
            tb[s].reshape(P, fpl),
            pb[s].reshape(P, fpl),
            pv[s].reshape(P, fpl),
        )
        # tile-major: per partition, each tile's 5 stream-chunks contiguous
        parts = []
        off = 0
        for fd in widths:
            for st in streams:
                parts.append(st[:, off : off + fd])
            off += fd
        big = np.concatenate(parts, axis=1)
        in_maps.append({"big": np.ascontiguousarray(big), "consts": consts})
    return in_maps


def _combine(core_outs, core_sums, widths=TILE_WIDTHS, n=N):
    """core_outs: [P, 3*n_tiles+n_dsubs] accumulators per core.
    core_sums: [2, MM] column sums per core (row 0 S_up, row 1 S_vx)."""
    n_tiles = len(widths)
    n_dsubs = _n_dsubs(widths)
    s_c = s_rx = s_sq = s_vd = s_up = s_vx = 0.0
    for o, ss in zip(core_outs, core_sums):
        o = np.asarray(o, dtype=np.float64)
        ss = np.asarray(ss, dtype=np.float64)
        s_c += o[:, 0:n_tiles].sum()
        s_rx += o[:, n_tiles : 2 * n_tiles].sum()
        s_sq += o[:, 2 * n_tiles : 3 * n_tiles].sum()
        s_vd += o[:, 3 * n_tiles : 3 * n_tiles + n_dsubs].sum()
        s_up += ss[0].sum()
        s_vx += ss[1].sum()

    s_w = 2.0 * s_up - 2.0 * s_c          # sum(up) - sum(lo)
    center_loss = s_sq / n
    width_loss = s_w / n
    valid_penalty = s_vd / n
    direction_penalty = s_rx - s_vx
    total = (
        center_loss * 10.0
        + 0.1 * width_loss
        + 10.0 * valid_penalty
        + 0.5 * direction_penalty / n
    )
    return np.float32(total)


def _run(inputs, trace=False):
    """Run the SPMD kernel; returns (scalar_result, BassKernelResults)."""
    from concourse.bass_utils import run_bass_kernel_spmd

    nc = _get_nc()
    in_maps = _shard(inputs)
    res = run_bass_kernel_spmd(
        nc, in_maps, core_ids=list(range(N_CORES)), trace=trace
    )
    core_outs = [res.results[c]["out"] for c in range(N_CORES)]
    core_sums = [res.results[c]["sums"] for c in range(N_CORES)]
    return _combine(core_outs, core_sums), res


def kernel(**inputs) -> np.ndarray:
    result, _ = _run(inputs, trace=False)
    return result


# revision 11
# speedup vs baseline: 1.1243x; 1.1243x over previous
"""Trainium2 Bass kernel for the interval-prediction custom loss.

total = 10*mean((t - (l+u)/2)^2) + 0.1*mean(u-l) + 10*mean(relu(l-u))
        + 0.5*sum(where(pv==0, relu(c-p), relu(p-c)))/N        with c=(l+u)/2

Strategy: pure data parallel over N across 8 NeuronCores; host does only the
tiny final scalar reduction in float64.

Engine plan (v3). Measured facts from v1/v2 traces: DVE tensor_tensor runs
2x and tensor_scalar 4x at bf16, but ANY DVE op with an accumulator
(tensor_scalar+accum, scalar_tensor_tensor, tensor_tensor_reduce) drops to
1x; ACT passes cost ~(fd+352)/1.2 ns regardless of dtype; PE matmuls cost
~250-450 ns per 512-col chunk. So v3 keeps the DVE on pure fast-mode ops
with no accumulators, and distributes the five reductions:

  DVE:    H = lo + up            (TT 2x)
          c = 0.5*H              (TS 4x)
          x = c - p              (TT 2x)
          e = c - t              (TT 2x)
          q = v * x              (TT 2x)
          rxt = max(x, 0)        (TS 4x)
  GPSIMD: w = lo - up            (TT, runs concurrent with DVE)
  ACT:    Square(e) accum -> S_sq
          Relu(w)   accum -> S_vd
  PE:     ones^T * w   -> PSUM[1,512]  (S_w partial column sums)
          ones^T * rxt -> PSUM[1,512]  (S_rx)
          ones^T * q   -> PSUM[1,512]  (S_vx)

Host combine: width = -S_w, direction = S_rx - S_vx.

All five streams are bf16 (int64 pv is 0/1 so the cast is exact); all
accumulation is fp32. Tile widths are multiples of 512 so each PE matmul
chunk is exactly 512 (one PSUM bank). The [lo,up] pair is a separate SBUF
tile from [t,p,v] so H/w start as soon as the first DMA lands.
"""

import sys

if "/opt/trn_rl_repo" not in sys.path:
    sys.path.insert(0, "/opt/trn_rl_repo")

import numpy as np

N = 8388608
N_CORES = 8
P = 128
NP_PER_CORE = N // N_CORES            # 1048576
FPL = NP_PER_CORE // P                # 8192 elements per partition lane
TILE_WIDTHS = (1024, 2048, 2048, 2048, 1024)
assert sum(TILE_WIDTHS) == FPL
assert all(w % 512 == 0 for w in TILE_WIDTHS)
MM = 512                              # matmul moving chunk / PSUM bank width

_NC_CACHE = {}


def _build(fpl=FPL, widths=TILE_WIDTHS):
    """Build the per-core Bass program (identical on all cores)."""
    from concourse import bacc, mybir
    from concourse.tile import TileContext

    assert sum(widths) == fpl
    n_tiles = len(widths)

    f32 = mybir.dt.float32
    bf16 = mybir.dt.bfloat16
    Alu = mybir.AluOpType
    Act = mybir.ActivationFunctionType

    nc = bacc.Bacc(trn_type="TRN2")
    big = nc.declare_dram_parameter("big", [P, 5 * fpl], bf16, isOutput=False)
    consts = nc.declare_dram_parameter("consts", [P, 1], bf16, isOutput=False)
    # accumulator columns: S_sq (n_tiles) | S_vd (n_tiles)
    out = nc.declare_dram_parameter("out", [P, 2 * n_tiles], f32, isOutput=True)
    # rows: S_w | S_rx | S_vx column sums
    sums = nc.declare_dram_parameter("sums", [3, MM], f32, isOutput=True)

    with TileContext(nc) as tc:
        with (
            tc.tile_pool(name="ioa", bufs=3) as ioa_pool,
            tc.tile_pool(name="iob", bufs=3) as iob_pool,
            tc.tile_pool(name="mid", bufs=3) as mid_pool,
            tc.tile_pool(name="acc", bufs=1) as acc_pool,
            tc.tile_pool(name="pss", bufs=1, space="PSUM") as pss_pool,
        ):
            acc_act = acc_pool.tile([P, 2 * n_tiles], f32, tag="acc_act")
            sw_sb = acc_pool.tile([1, MM], f32, tag="sw_sb")
            srx_sb = acc_pool.tile([1, MM], f32, tag="srx_sb")
            svx_sb = acc_pool.tile([1, MM], f32, tag="svx_sb")
            ps_w = pss_pool.tile([1, MM], f32, tag="ps_w")
            ps_rx = pss_pool.tile([1, MM], f32, tag="ps_rx")
            ps_vx = pss_pool.tile([1, MM], f32, tag="ps_vx")

            const_t = acc_pool.tile([P, 1], bf16, tag="consts")
            onesv = const_t[:, 0:1]

            off = 0
            first = [True, True, True]
            n_chunks = fpl // MM
            done_chunks = 0
            for j, tw in enumerate(widths):
                big_a = ioa_pool.tile([P, 2, tw], bf16, tag="biga", name=f"biga{j}")
                big_b = iob_pool.tile([P, 3, tw], bf16, tag="bigb", name=f"bigb{j}")
                src = big[:, off : off + 5 * tw].rearrange("p (s f) -> p s f", s=5)
                nc.sync.dma_start(out=big_a, in_=src[:, 0:2, :])
                if j == 0:
                    # consts needed only by the PE; issue after the first
                    # compute-critical DMA
                    nc.sync.dma_start(out=const_t, in_=consts[:, :])
                nc.sync.dma_start(out=big_b, in_=src[:, 2:5, :])
                off += 5 * tw

                lo = big_a[:, 0, :]
                up = big_a[:, 1, :]
                t_t = big_b[:, 0, :]
                p_t = big_b[:, 1, :]
                v_t = big_b[:, 2, :]

                H = mid_pool.tile([P, tw], bf16, tag="H", name=f"H{j}")
                c = mid_pool.tile([P, tw], bf16, tag="c", name=f"c{j}")
                x = mid_pool.tile([P, tw], bf16, tag="x", name=f"x{j}")
                e = mid_pool.tile([P, tw], bf16, tag="e", name=f"e{j}")
                q = mid_pool.tile([P, tw], bf16, tag="q", name=f"q{j}")
                rxt = mid_pool.tile([P, tw], bf16, tag="rxt", name=f"rxt{j}")
                w = mid_pool.tile([P, tw], bf16, tag="w", name=f"w{j}")
                ja = mid_pool.tile([P, tw], bf16, tag="ja", name=f"ja{j}")
                jb = mid_pool.tile([P, tw], bf16, tag="jb", name=f"jb{j}")

                # --- GPSIMD (concurrent with DVE): w = lo - up ---
                nc.gpsimd.tensor_sub(out=w, in0=lo, in1=up)

                # --- DVE: fast-mode ops only ---
                nc.vector.tensor_add(out=H, in0=lo, in1=up)
                nc.vector.tensor_scalar(
                    out=c, in0=H, scalar1=0.5, scalar2=None, op0=Alu.mult
                )
                nc.vector.tensor_sub(out=x, in0=c, in1=p_t)
                nc.vector.tensor_sub(out=e, in0=c, in1=t_t)
                nc.vector.tensor_mul(out=q, in0=v_t, in1=x)
                nc.vector.tensor_scalar(
                    out=rxt, in0=x, scalar1=0.0, scalar2=None, op0=Alu.max
                )

                # --- ACT: the two nonlinear accumulations ---
                nc.scalar.activation(
                    out=ja, in_=e, func=Act.Square,
                    accum_out=acc_act[:, j : j + 1],
                )
                nc.scalar.activation(
                    out=jb, in_=w, func=Act.Relu,
                    accum_out=acc_act[:, n_tiles + j : n_tiles + j + 1],
                )

                # --- PE: column-sum accumulators (stationary = ones, loaded
                # once) ---
                for ci, ch in enumerate(range(0, tw, MM)):
                    is_last = done_chunks + ci == n_chunks - 1
                    nc.tensor.matmul(
                        ps_w, onesv, w[:, ch : ch + MM],
                        start=first[0], stop=is_last,
                    )
                    first[0] = False
                for ci, ch in enumerate(range(0, tw, MM)):
                    is_last = done_chunks + ci == n_chunks - 1
                    nc.tensor.matmul(
                        ps_rx, onesv, rxt[:, ch : ch + MM],
                        start=first[1], stop=is_last,
                    )
                    first[1] = False
                for ci, ch in enumerate(range(0, tw, MM)):
                    is_last = done_chunks + ci == n_chunks - 1
                    nc.tensor.matmul(
                        ps_vx, onesv, q[:, ch : ch + MM],
                        start=first[2], stop=is_last,
                    )
                    first[2] = False
                done_chunks += tw // MM

            # PSUM -> SBUF -> DRAM for the column sums
            nc.scalar.activation(out=sw_sb[:, :], in_=ps_w, func=Act.Copy)
            nc.scalar.activation(out=srx_sb[:, :], in_=ps_rx, func=Act.Copy)
            nc.scalar.activation(out=svx_sb[:, :], in_=ps_vx, func=Act.Copy)

            nc.sync.dma_start(out=out[:, :], in_=acc_act)
            nc.sync.dma_start(out=sums[0:1, :], in_=sw_sb)
            nc.sync.dma_start(out=sums[1:2, :], in_=srx_sb)
            nc.sync.dma_start(out=sums[2:3, :], in_=svx_sb)

    nc.compile()
    return nc


def _get_nc():
    key = (FPL, TILE_WIDTHS)
    if key not in _NC_CACHE:
        _NC_CACHE[key] = _build()
    return _NC_CACHE[key]


def _make_consts():
    import ml_dtypes

    return np.ones((P, 1), dtype=ml_dtypes.bfloat16)


def _shard(inputs, fpl=FPL, widths=TILE_WIDTHS, n_cores=N_CORES):
    import ml_dtypes

    bf = ml_dtypes.bfloat16
    n = n_cores * P * fpl
    pred = np.asarray(inputs["pred"])
    targ = np.asarray(inputs["target"]).reshape(n)
    prev = np.asarray(inputs["prev_pci"]).reshape(n)
    # int64 is unsupported on-device; values are 0/1 so a bf16 cast is exact.
    pv = np.asarray(inputs["pv_values"]).astype(bf).reshape(n)

    lo = pred[:, 0].astype(bf)
    up = pred[:, 1].astype(bf)
    tb = targ.astype(bf)
    pb = prev.astype(bf)

    consts = _make_consts()
    np_per_core = P * fpl

    in_maps = []
    for cix in range(n_cores):
        s = slice(cix * np_per_core, (cix + 1) * np_per_core)
        streams = (
            lo[s].reshape(P, fpl),
            up[s].reshape(P, fpl),
            tb[s].reshape(P, fpl),
            pb[s].reshape(P, fpl),
            pv[s].reshape(P, fpl),
        )
        # tile-major: per partition, each tile's 5 stream-chunks contiguous
        parts = []
        off = 0
        for fd in widths:
            for st in streams:
                parts.append(st[:, off : off + fd])
            off += fd
        big = np.concatenate(parts, axis=1)
        in_maps.append({"big": np.ascontiguousarray(big), "consts": consts})
    return in_maps


def _combine(core_outs, core_sums, widths=TILE_WIDTHS, n=N):
    """core_outs: [P, 2*n_tiles] ACT accumulators (S_sq | S_vd) per core.
    core_sums: [3, MM] column sums per core (S_w=sum(lo-up) | S_rx | S_vx)."""
    n_tiles = len(widths)
    s_sq = s_vd = s_w = s_rx = s_vx = 0.0
    for o, ss in zip(core_outs, core_sums):
        o = np.asarray(o, dtype=np.float64)
        ss = np.asarray(ss, dtype=np.float64)
        s_sq += o[:, 0:n_tiles].sum()
        s_vd += o[:, n_tiles : 2 * n_tiles].sum()
        s_w += ss[0].sum()
        s_rx += ss[1].sum()
        s_vx += ss[2].sum()

    center_loss = s_sq / n
    width_loss = -s_w / n                  # sum(up - lo) = -sum(lo - up)
    valid_penalty = s_vd / n
    direction_penalty = s_rx - s_vx
    total = (
        center_loss * 10.0
        + 0.1 * width_loss
        + 10.0 * valid_penalty
        + 0.5 * direction_penalty / n
    )
    return np.float32(total)


def _run(inputs, trace=False):
    """Run the SPMD kernel; returns (scalar_result, BassKernelResults)."""
    from concourse.bass_utils import run_bass_kernel_spmd

    nc = _get_nc()
    in_maps = _shard(inputs)
    res = run_bass_kernel_spmd(
        nc, in_maps, core_ids=list(range(N_CORES)), trace=trace
    )
    core_outs = [res.results[c]["out"] for c in range(N_CORES)]
    core_sums = [res.results[c]["sums"] for c in range(N_CORES)]
    return _combine(core_outs, core_sums), res


def kernel(**inputs) -> np.ndarray:
    result, _ = _run(inputs, trace=False)
    return result


# revision 12
# speedup vs baseline: 1.1999x; 1.0672x over previous
"""Trainium2 Bass kernel for the interval-prediction custom loss.

total = 10*mean((t - (l+u)/2)^2) + 0.1*mean(u-l) + 10*mean(relu(l-u))
        + 0.5*sum(where(pv==0, relu(c-p), relu(p-c)))/N        with c=(l+u)/2

Strategy: pure data parallel over N across 8 NeuronCores; host does only the
tiny final scalar reduction in float64.

Engine plan (v4). Measured facts from earlier traces: DVE tensor_tensor runs
2x and tensor_scalar 4x at bf16, but ANY DVE op with an accumulator drops to
1x; GPSIMD elementwise ops contend for the DVE's SBUF port and degrade DVE
throughput ~2x, so GPSIMD stays idle; ACT costs ~(fd+352)/1.2 ns per pass;
PE matmuls ~250-450 ns per 512-col chunk. Assignment:

  DVE:    H = lo + up            (TT 2x)
          c = 0.5*H              (TS 4x)
          x = c - p              (TT 2x)
          e = c - t              (TT 2x)
          q = v * x              (TT 2x)
          rxt = max(x, 0)        (TS 4x)
  PE:     d = I*lo + (-I)*up -> PSUM        (identity matmuls)
          ones^T * rxt -> PSUM[1,512]       (S_rx column sums)
          ones^T * q   -> PSUM[1,512]       (S_vx)
  ACT:    Square(e)    accum -> S_sq
          Relu(d)      accum -> S_vd   (= sum relu(lo-up))
          Relu(-d)     accum -> S_vd2  (= sum relu(up-lo))

Host combine: width sum = S_vd2 - S_vd, direction = S_rx - S_vx.

All five streams are bf16 (int64 pv is 0/1 so the cast is exact); all
accumulation is fp32. Tile widths are multiples of 512 so each PE matmul
chunk is exactly 512 (one PSUM bank). The [lo,up] pair is a separate SBUF
tile from [t,p,v] so H and the d-matmuls start as soon as the first DMA
lands.
"""

import sys

if "/opt/trn_rl_repo" not in sys.path:
    sys.path.insert(0, "/opt/trn_rl_repo")

import numpy as np

N = 8388608
N_CORES = 8
P = 128
NP_PER_CORE = N // N_CORES            # 1048576
FPL = NP_PER_CORE // P                # 8192 elements per partition lane
TILE_WIDTHS = (1024, 2048, 2048, 2048, 1024)
assert sum(TILE_WIDTHS) == FPL
assert all(w % 512 == 0 for w in TILE_WIDTHS)
MM = 512                              # matmul moving chunk / PSUM bank width
DSUB = 1024                           # PSUM d-subtile width (2 banks)

_NC_CACHE = {}


def _dsub_widths(tw):
    out = []
    off = 0
    while off < tw:
        w = min(DSUB, tw - off)
        out.append(w)
        off += w
    return out


def _n_dsubs(widths):
    return sum(len(_dsub_widths(tw)) for tw in widths)


def _build(fpl=FPL, widths=TILE_WIDTHS):
    """Build the per-core Bass program (identical on all cores)."""
    from concourse import bacc, mybir
    from concourse.tile import TileContext

    assert sum(widths) == fpl
    n_tiles = len(widths)
    n_dsubs = _n_dsubs(widths)

    f32 = mybir.dt.float32
    bf16 = mybir.dt.bfloat16
    Alu = mybir.AluOpType
    Act = mybir.ActivationFunctionType

    nc = bacc.Bacc(trn_type="TRN2")
    big = nc.declare_dram_parameter("big", [P, 5 * fpl], bf16, isOutput=False)
    # columns: [0:128) identity, [128:256) -identity, [256] ones
    consts = nc.declare_dram_parameter("consts", [P, 257], bf16, isOutput=False)
    # accumulator columns: S_sq (n_tiles) | S_vd (n_dsubs) | S_vd2 (n_dsubs)
    out = nc.declare_dram_parameter(
        "out", [P, n_tiles + 2 * n_dsubs], f32, isOutput=True
    )
    # rows: S_rx | S_vx column sums
    sums = nc.declare_dram_parameter("sums", [2, MM], f32, isOutput=True)

    with TileContext(nc) as tc:
        with (
            tc.tile_pool(name="ioa", bufs=3) as ioa_pool,
            tc.tile_pool(name="iob", bufs=3) as iob_pool,
            tc.tile_pool(name="mid", bufs=3) as mid_pool,
            tc.tile_pool(name="acc", bufs=1) as acc_pool,
            tc.tile_pool(name="psd", bufs=2, space="PSUM") as psd_pool,
            tc.tile_pool(name="pss", bufs=1, space="PSUM") as pss_pool,
        ):
            acc_act = acc_pool.tile([P, n_tiles + 2 * n_dsubs], f32, tag="acc_act")
            srx_sb = acc_pool.tile([1, MM], f32, tag="srx_sb")
            svx_sb = acc_pool.tile([1, MM], f32, tag="svx_sb")
            ps_rx = pss_pool.tile([1, MM], f32, tag="ps_rx")
            ps_vx = pss_pool.tile([1, MM], f32, tag="ps_vx")

            const_t = acc_pool.tile([P, 257], bf16, tag="consts")
            ident = const_t[:, 0:128]
            nident = const_t[:, 128:256]
            onesv = const_t[:, 256:257]

            off = 0
            dix = 0
            first = [True, True]
            n_chunks = fpl // MM
            done_chunks = 0
            for j, tw in enumerate(widths):
                big_a = ioa_pool.tile([P, 2, tw], bf16, tag="biga", name=f"biga{j}")
                big_b = iob_pool.tile([P, 3, tw], bf16, tag="bigb", name=f"bigb{j}")
                src = big[:, off : off + 5 * tw].rearrange("p (s f) -> p s f", s=5)
                nc.sync.dma_start(out=big_a, in_=src[:, 0:2, :])
                if j == 0:
                    # consts needed only by the PE; issue after the first
                    # compute-critical DMA
                    nc.sync.dma_start(out=const_t, in_=consts[:, :])
                nc.sync.dma_start(out=big_b, in_=src[:, 2:5, :])
                off += 5 * tw

                lo = big_a[:, 0, :]
                up = big_a[:, 1, :]
                t_t = big_b[:, 0, :]
                p_t = big_b[:, 1, :]
                v_t = big_b[:, 2, :]

                H = mid_pool.tile([P, tw], bf16, tag="H", name=f"H{j}")
                c = mid_pool.tile([P, tw], bf16, tag="c", name=f"c{j}")
                x = mid_pool.tile([P, tw], bf16, tag="x", name=f"x{j}")
                e = mid_pool.tile([P, tw], bf16, tag="e", name=f"e{j}")
                q = mid_pool.tile([P, tw], bf16, tag="q", name=f"q{j}")
                rxt = mid_pool.tile([P, tw], bf16, tag="rxt", name=f"rxt{j}")
                ja = mid_pool.tile([P, tw], bf16, tag="ja", name=f"ja{j}")

                # --- DVE: fast-mode ops only, no accumulators ---
                nc.vector.tensor_add(out=H, in0=lo, in1=up)
                nc.vector.tensor_scalar(
                    out=c, in0=H, scalar1=0.5, scalar2=None, op0=Alu.mult
                )
                nc.vector.tensor_sub(out=x, in0=c, in1=p_t)
                nc.vector.tensor_sub(out=e, in0=c, in1=t_t)
                nc.vector.tensor_mul(out=q, in0=v_t, in1=x)
                nc.vector.tensor_scalar(
                    out=rxt, in0=x, scalar1=0.0, scalar2=None, op0=Alu.max
                )

                # --- ACT: sum(e^2) ---
                nc.scalar.activation(
                    out=ja, in_=e, func=Act.Square,
                    accum_out=acc_act[:, j : j + 1],
                )

                # --- PE: d = lo - up into PSUM; ACT relu-sums both signs ---
                doff = 0
                for dw in _dsub_widths(tw):
                    d_ps = psd_pool.tile([P, dw], f32, tag="d_ps", name=f"d{dix}")
                    for ch in range(0, dw, MM):
                        nc.tensor.matmul(
                            d_ps[:, ch : ch + MM],
                            ident,
                            lo[:, doff + ch : doff + ch + MM],
                            start=True, stop=False,
                        )
                    for ch in range(0, dw, MM):
                        nc.tensor.matmul(
                            d_ps[:, ch : ch + MM],
                            nident,
                            up[:, doff + ch : doff + ch + MM],
                            start=False, stop=True,
                        )
                    jd = mid_pool.tile([P, dw], bf16, tag="jd", name=f"jd{dix}")
                    nc.scalar.activation(
                        out=jd, in_=d_ps, func=Act.Relu,
                        accum_out=acc_act[:, n_tiles + dix : n_tiles + dix + 1],
                    )
                    je = mid_pool.tile([P, dw], bf16, tag="je", name=f"je{dix}")
                    nc.scalar.activation(
                        out=je, in_=d_ps, func=Act.Relu, scale=-1.0,
                        accum_out=acc_act[
                            :, n_tiles + n_dsubs + dix : n_tiles + n_dsubs + dix + 1
                        ],
                    )
                    doff += dw
                    dix += 1

                # --- PE: column-sum accumulators (ones stationary) ---
                for ci, ch in enumerate(range(0, tw, MM)):
                    is_last = done_chunks + ci == n_chunks - 1
                    nc.tensor.matmul(
                        ps_rx, onesv, rxt[:, ch : ch + MM],
                        start=first[0], stop=is_last,
                    )
                    first[0] = False
                for ci, ch in enumerate(range(0, tw, MM)):
                    is_last = done_chunks + ci == n_chunks - 1
                    nc.tensor.matmul(
                        ps_vx, onesv, q[:, ch : ch + MM],
                        start=first[1], stop=is_last,
                    )
                    first[1] = False
                done_chunks += tw // MM

            assert dix == n_dsubs

            # PSUM -> SBUF -> DRAM for the column sums
            nc.scalar.activation(out=srx_sb[:, :], in_=ps_rx, func=Act.Copy)
            nc.scalar.activation(out=svx_sb[:, :], in_=ps_vx, func=Act.Copy)

            nc.sync.dma_start(out=out[:, :], in_=acc_act)
            nc.sync.dma_start(out=sums[0:1, :], in_=srx_sb)
            nc.sync.dma_start(out=sums[1:2, :], in_=svx_sb)

    nc.compile()
    return nc


def _get_nc():
    key = (FPL, TILE_WIDTHS)
    if key not in _NC_CACHE:
        _NC_CACHE[key] = _build()
    return _NC_CACHE[key]


def _make_consts():
    import ml_dtypes

    bf = ml_dtypes.bfloat16
    consts = np.zeros((P, 257), dtype=bf)
    consts[:, 0:128] = np.eye(P, dtype=bf)
    consts[:, 128:256] = -np.eye(P, dtype=bf)
    consts[:, 256] = bf(1.0)
    return consts


def _shard(inputs, fpl=FPL, widths=TILE_WIDTHS, n_cores=N_CORES):
    import ml_dtypes

    bf = ml_dtypes.bfloat16
    n = n_cores * P * fpl
    pred = np.asarray(inputs["pred"])
    targ = np.asarray(inputs["target"]).reshape(n)
    prev = np.asarray(inputs["prev_pci"]).reshape(n)
    # int64 is unsupported on-device; values are 0/1 so a bf16 cast is exact.
    pv = np.asarray(inputs["pv_values"]).astype(bf).reshape(n)

    lo = pred[:, 0].astype(bf)
    up = pred[:, 1].astype(bf)
    tb = targ.astype(bf)
    pb = prev.astype(bf)

    consts = _make_consts()
    np_per_core = P * fpl

    in_maps = []
    for cix in range(n_cores):
        s = slice(cix * np_per_core, (cix + 1) * np_per_core)
        streams = (
            lo[s].reshape(P, fpl),
            up[s].reshape(P, fpl),
            tb[s].reshape(P, fpl),
            pb[s].reshape(P, fpl),
            pv[s].reshape(P, fpl),
        )
        # tile-major: per partition, each tile's 5 stream-chunks contiguous
        parts = []
        off = 0
        for fd in widths:
            for st in streams:
                parts.append(st[:, off : off + fd])
            off += fd
        big = np.concatenate(parts, axis=1)
        in_maps.append({"big": np.ascontiguousarray(big), "consts": consts})
    return in_maps


def _combine(core_outs, core_sums, widths=TILE_WIDTHS, n=N):
    """core_outs: [P, n_tiles + 2*n_dsubs] ACT accumulators per core
    (S_sq | S_vd | S_vd2).  core_sums: [2, MM] column sums (S_rx | S_vx)."""
    n_tiles = len(widths)
    n_dsubs = _n_dsubs(widths)
    s_sq = s_vd = s_vd2 = s_rx = s_vx = 0.0
    for o, ss in zip(core_outs, core_sums):
        o = np.asarray(o, dtype=np.float64)
        ss = np.asarray(ss, dtype=np.float64)
        s_sq += o[:, 0:n_tiles].sum()
        s_vd += o[:, n_tiles : n_tiles + n_dsubs].sum()
        s_vd2 += o[:, n_tiles + n_dsubs : n_tiles + 2 * n_dsubs].sum()
        s_rx += ss[0].sum()
        s_vx += ss[1].sum()

    center_loss = s_sq / n
    width_loss = (s_vd2 - s_vd) / n        # sum(up-lo) = relu(up-lo)-relu(lo-up)
    valid_penalty = s_vd / n
    direction_penalty = s_rx - s_vx
    total = (
        center_loss * 10.0
        + 0.1 * width_loss
        + 10.0 * valid_penalty
        + 0.5 * direction_penalty / n
    )
    return np.float32(total)


def _run(inputs, trace=False):
    """Run the SPMD kernel; returns (scalar_result, BassKernelResults)."""
    from concourse.bass_utils import run_bass_kernel_spmd

    nc = _get_nc()
    in_maps = _shard(inputs)
    res = run_bass_kernel_spmd(
        nc, in_maps, core_ids=list(range(N_CORES)), trace=trace
    )
    core_outs = [res.results[c]["out"] for c in range(N_CORES)]
    core_sums = [res.results[c]["sums"] for c in range(N_CORES)]
    return _combine(core_outs, core_sums), res


def kernel(**inputs) -> np.ndarray:
    result, _ = _run(inputs, trace=False)
    return result


# revision 17
# speedup vs baseline: 1.2542x; 1.0453x over previous
"""Trainium2 Bass kernel for the interval-prediction custom loss.

total = 10*mean((t - (l+u)/2)^2) + 0.1*mean(u-l) + 10*mean(relu(l-u))
        + 0.5*sum(where(pv==0, relu(c-p), relu(p-c)))/N        with c=(l+u)/2

Strategy: pure data parallel over N across 8 NeuronCores; host does only the
tiny final scalar reduction in float64.

Engine plan (v4). Measured facts from earlier traces: DVE tensor_tensor runs
2x and tensor_scalar 4x at bf16, but ANY DVE op with an accumulator drops to
1x; GPSIMD elementwise ops contend for the DVE's SBUF port and degrade DVE
throughput ~2x, so GPSIMD stays idle; ACT costs ~(fd+352)/1.2 ns per pass;
PE matmuls ~250-450 ns per 512-col chunk. Assignment:

  DVE:    H = lo + up            (TT 2x)
          c = 0.5*H              (TS 4x)
          x = c - p              (TT 2x)
          e = c - t              (TT 2x)
          q = v * x              (TT 2x)
          rxt = max(x, 0)        (TS 4x)
  PE:     d = I*lo + (-I)*up -> PSUM        (identity matmuls)
          ones^T * rxt -> PSUM[1,512]       (S_rx column sums)
          ones^T * q   -> PSUM[1,512]       (S_vx)
  ACT:    Square(e)    accum -> S_sq
          Relu(d)      accum -> S_vd   (= sum relu(lo-up))
          Relu(-d)     accum -> S_vd2  (= sum relu(up-lo))

Host combine: width sum = S_vd2 - S_vd, direction = S_rx - S_vx.

All five streams are bf16 (int64 pv is 0/1 so the cast is exact); all
accumulation is fp32. Tile widths are multiples of 512 so each PE matmul
chunk is exactly 512 (one PSUM bank). The [lo,up] pair is a separate SBUF
tile from [t,p,v] so H and the d-matmuls start as soon as the first DMA
lands.
"""

import sys

if "/opt/trn_rl_repo" not in sys.path:
    sys.path.insert(0, "/opt/trn_rl_repo")

import numpy as np

N = 8388608
N_CORES = 8
P = 128
NP_PER_CORE = N // N_CORES            # 1048576
FPL = NP_PER_CORE // P                # 8192 elements per partition lane
TILE_WIDTHS = (1024, 2048, 2048, 2048, 1024)
assert sum(TILE_WIDTHS) == FPL
assert all(w % 512 == 0 for w in TILE_WIDTHS)
MM = 512                              # matmul moving chunk / PSUM bank width
DSUB = 1024                           # PSUM d-subtile width (2 banks)

_NC_CACHE = {}


def _dsub_widths(tw):
    out = []
    off = 0
    while off < tw:
        w = min(DSUB, tw - off)
        out.append(w)
        off += w
    return out


def _n_dsubs(widths):
    return sum(len(_dsub_widths(tw)) for tw in widths)


def _build(fpl=FPL, widths=TILE_WIDTHS):
    """Build the per-core Bass program (identical on all cores)."""
    from concourse import bacc, mybir
    from concourse.tile import TileContext

    assert sum(widths) == fpl
    n_tiles = len(widths)
    n_dsubs = _n_dsubs(widths)

    f32 = mybir.dt.float32
    bf16 = mybir.dt.bfloat16
    Alu = mybir.AluOpType
    Act = mybir.ActivationFunctionType

    nc = bacc.Bacc(trn_type="TRN2")
    big = nc.declare_dram_parameter("big", [P, 5 * fpl], bf16, isOutput=False)
    # columns: [0:128) identity, [128:256) -identity, [256] ones
    consts = nc.declare_dram_parameter("consts", [P, 257], bf16, isOutput=False)
    # accumulator columns: S_sq (n_tiles) | S_vd (n_dsubs) | S_vd2 (n_dsubs)
    out = nc.declare_dram_parameter(
        "out", [P, n_tiles + 2 * n_dsubs], f32, isOutput=True
    )
    # rows: S_rx | S_vx column sums
    sums = nc.declare_dram_parameter("sums", [2, MM], f32, isOutput=True)

    with TileContext(nc) as tc:
        with (
            tc.tile_pool(name="ioa", bufs=4) as ioa_pool,
            tc.tile_pool(name="iob", bufs=4) as iob_pool,
            tc.tile_pool(name="mid", bufs=4) as mid_pool,
            tc.tile_pool(name="jnk", bufs=2) as jnk_pool,
            tc.tile_pool(name="acc", bufs=1) as acc_pool,
            tc.tile_pool(name="psd", bufs=2, space="PSUM") as psd_pool,
            tc.tile_pool(name="pss", bufs=1, space="PSUM") as pss_pool,
        ):
            acc_act = acc_pool.tile([P, n_tiles + 2 * n_dsubs], f32, tag="acc_act")
            srx_sb = acc_pool.tile([1, MM], f32, tag="srx_sb")
            svx_sb = acc_pool.tile([1, MM], f32, tag="svx_sb")
            ps_rx = pss_pool.tile([1, MM], f32, tag="ps_rx")
            ps_vx = pss_pool.tile([1, MM], f32, tag="ps_vx")

            const_t = acc_pool.tile([P, 257], bf16, tag="consts")
            ident = const_t[:, 0:128]
            nident = const_t[:, 128:256]
            onesv = const_t[:, 256:257]

            def ones_mms(tw, rxt, q, done_before):
                """Column-sum matmuls for tile jj (emitted one tile late so
                the in-order PE queue never waits on the DVE)."""
                for ci, ch in enumerate(range(0, tw, MM)):
                    is_last = done_before + ci == n_chunks - 1
                    nc.tensor.matmul(
                        ps_rx, onesv, rxt[:, ch : ch + MM],
                        start=first[0], stop=is_last,
                    )
                    first[0] = False
                for ci, ch in enumerate(range(0, tw, MM)):
                    is_last = done_before + ci == n_chunks - 1
                    nc.tensor.matmul(
                        ps_vx, onesv, q[:, ch : ch + MM],
                        start=first[1], stop=is_last,
                    )
                    first[1] = False

            off = 0
            dix = 0
            first = [True, True]
            n_chunks = fpl // MM
            done_chunks = 0
            pending = None  # (j, tw, rxt, q, done_before) of previous tile
            for j, tw in enumerate(widths):
                big_a = ioa_pool.tile([P, 2, tw], bf16, tag="biga", name=f"biga{j}")
                big_b = iob_pool.tile([P, 3, tw], bf16, tag="bigb", name=f"bigb{j}")
                src = big[:, off : off + 5 * tw].rearrange("p (s f) -> p s f", s=5)
                nc.sync.dma_start(out=big_a, in_=src[:, 0:2, :])
                if j == 0:
                    # consts needed only by the PE; issue after the first
                    # compute-critical DMA
                    nc.sync.dma_start(out=const_t, in_=consts[:, :])
                nc.sync.dma_start(out=big_b, in_=src[:, 2:5, :])
                off += 5 * tw

                lo = big_a[:, 0, :]
                up = big_a[:, 1, :]
                t_t = big_b[:, 0, :]
                p_t = big_b[:, 1, :]
                v_t = big_b[:, 2, :]

                H = mid_pool.tile([P, tw], bf16, tag="H", name=f"H{j}")
                c = mid_pool.tile([P, tw], bf16, tag="c", name=f"c{j}")
                x = mid_pool.tile([P, tw], bf16, tag="x", name=f"x{j}")
                e = mid_pool.tile([P, tw], bf16, tag="e", name=f"e{j}")
                q = mid_pool.tile([P, tw], bf16, tag="q", name=f"q{j}")
                rxt = mid_pool.tile([P, tw], bf16, tag="rxt", name=f"rxt{j}")
                ja = jnk_pool.tile([P, tw], bf16, tag="ja", name=f"ja{j}")

                # --- DVE: fast-mode ops only, no accumulators ---
                nc.vector.tensor_add(out=H, in0=lo, in1=up)
                nc.vector.tensor_scalar(
                    out=c, in0=H, scalar1=0.5, scalar2=None, op0=Alu.mult
                )
                nc.vector.tensor_sub(out=x, in0=c, in1=p_t)
                nc.vector.tensor_sub(out=e, in0=c, in1=t_t)
                nc.vector.tensor_mul(out=q, in0=v_t, in1=x)
                nc.vector.tensor_scalar(
                    out=rxt, in0=x, scalar1=0.0, scalar2=None, op0=Alu.max
                )

                # --- PE: d = lo - up into PSUM; ACT relu-sums both signs ---
                doff = 0
                for dw in _dsub_widths(tw):
                    d_ps = psd_pool.tile([P, dw], f32, tag="d_ps", name=f"d{dix}")
                    for ch in range(0, dw, MM):
                        nc.tensor.matmul(
                            d_ps[:, ch : ch + MM],
                            ident,
                            lo[:, doff + ch : doff + ch + MM],
                            start=True, stop=False,
                        )
                    for ch in range(0, dw, MM):
                        nc.tensor.matmul(
                            d_ps[:, ch : ch + MM],
                            nident,
                            up[:, doff + ch : doff + ch + MM],
                            start=False, stop=True,
                        )
                    jd = jnk_pool.tile([P, dw], bf16, tag="jd", name=f"jd{dix}")
                    nc.scalar.activation(
                        out=jd, in_=d_ps, func=Act.Relu,
                        accum_out=acc_act[:, n_tiles + dix : n_tiles + dix + 1],
                    )
                    je = jnk_pool.tile([P, dw], bf16, tag="je", name=f"je{dix}")
                    nc.scalar.activation(
                        out=je, in_=d_ps, func=Act.Relu, scale=-1.0,
                        accum_out=acc_act[
                            :, n_tiles + n_dsubs + dix : n_tiles + n_dsubs + dix + 1
                        ],
                    )
                    doff += dw
                    dix += 1

                # --- ACT: sum(e^2) ---
                nc.scalar.activation(
                    out=ja, in_=e, func=Act.Square,
                    accum_out=acc_act[:, j : j + 1],
                )

                # --- PE: column sums for the PREVIOUS tile ---
                if pending is not None:
                    ones_mms(*pending)
                pending = (tw, rxt, q, done_chunks)
                done_chunks += tw // MM

            ones_mms(*pending)
            assert dix == n_dsubs

            # PSUM -> SBUF -> DRAM for the column sums
            nc.scalar.activation(out=srx_sb[:, :], in_=ps_rx, func=Act.Copy)
            nc.scalar.activation(out=svx_sb[:, :], in_=ps_vx, func=Act.Copy)

            nc.sync.dma_start(out=out[:, :], in_=acc_act)
            nc.sync.dma_start(out=sums[0:1, :], in_=srx_sb)
            nc.sync.dma_start(out=sums[1:2, :], in_=svx_sb)

    nc.compile()
    return nc


def _get_nc():
    key = (FPL, TILE_WIDTHS)
    if key not in _NC_CACHE:
        _NC_CACHE[key] = _build()
    return _NC_CACHE[key]


def _make_consts():
    import ml_dtypes

    bf = ml_dtypes.bfloat16
    consts = np.zeros((P, 257), dtype=bf)
    consts[:, 0:128] = np.eye(P, dtype=bf)
    consts[:, 128:256] = -np.eye(P, dtype=bf)
    consts[:, 256] = bf(1.0)
    return consts


def _shard(inputs, fpl=FPL, widths=TILE_WIDTHS, n_cores=N_CORES):
    import ml_dtypes

    bf = ml_dtypes.bfloat16
    n = n_cores * P * fpl
    pred = np.asarray(inputs["pred"])
    targ = np.asarray(inputs["target"]).reshape(n)
    prev = np.asarray(inputs["prev_pci"]).reshape(n)
    # int64 is unsupported on-device; values are 0/1 so a bf16 cast is exact.
    pv = np.asarray(inputs["pv_values"]).astype(bf).reshape(n)

    lo = pred[:, 0].astype(bf)
    up = pred[:, 1].astype(bf)
    tb = targ.astype(bf)
    pb = prev.astype(bf)

    consts = _make_consts()
    np_per_core = P * fpl

    in_maps = []
    for cix in range(n_cores):
        s = slice(cix * np_per_core, (cix + 1) * np_per_core)
        streams = (
            lo[s].reshape(P, fpl),
            up[s].reshape(P, fpl),
            tb[s].reshape(P, fpl),
            pb[s].reshape(P, fpl),
            pv[s].reshape(P, fpl),
        )
        # tile-major: per partition, each tile's 5 stream-chunks contiguous
        parts = []
        off = 0
        for fd in widths:
            for st in streams:
                parts.append(st[:, off : off + fd])
            off += fd
        big = np.concatenate(parts, axis=1)
        in_maps.append({"big": np.ascontiguousarray(big), "consts": consts})
    return in_maps


def _combine(core_outs, core_sums, widths=TILE_WIDTHS, n=N):
    """core_outs: [P, n_tiles + 2*n_dsubs] ACT accumulators per core
    (S_sq | S_vd | S_vd2).  core_sums: [2, MM] column sums (S_rx | S_vx)."""
    n_tiles = len(widths)
    n_dsubs = _n_dsubs(widths)
    s_sq = s_vd = s_vd2 = s_rx = s_vx = 0.0
    for o, ss in zip(core_outs, core_sums):
        o = np.asarray(o, dtype=np.float64)
        ss = np.asarray(ss, dtype=np.float64)
        s_sq += o[:, 0:n_tiles].sum()
        s_vd += o[:, n_tiles : n_tiles + n_dsubs].sum()
        s_vd2 += o[:, n_tiles + n_dsubs : n_tiles + 2 * n_dsubs].sum()
        s_rx += ss[0].sum()
        s_vx += ss[1].sum()

    center_loss = s_sq / n
    width_loss = (s_vd2 - s_vd) / n        # sum(up-lo) = relu(up-lo)-relu(lo-up)
    valid_penalty = s_vd / n
    direction_penalty = s_rx - s_vx
    total = (
        center_loss * 10.0
        + 0.1 * width_loss
        + 10.0 * valid_penalty
        + 0.5 * direction_penalty / n
    )
    return np.float32(total)


def _run(inputs, trace=False):
    """Run the SPMD kernel; returns (scalar_result, BassKernelResults)."""
    from concourse.bass_utils import run_bass_kernel_spmd

    nc = _get_nc()
    in_maps = _shard(inputs)
    res = run_bass_kernel_spmd(
        nc, in_maps, core_ids=list(range(N_CORES)), trace=trace
    )
    core_outs = [res.results[c]["out"] for c in range(N_CORES)]
    core_sums = [res.results[c]["sums"] for c in range(N_CORES)]
    return _combine(core_outs, core_sums), res


def kernel(**inputs) -> np.ndarray:
    result, _ = _run(inputs, trace=False)
    return result
